# revision 1
# baseline (speedup 1.0000x reference)
"""Trainium2 Bass kernel for nn_AttentionKernelIntegral (linear attention).

Math (per batch b, head h):
    q = x @ Wq^T                      [N, 512]  (no norm)
    k = inorm(x @ Wk^T)               per-(n,h) mean/var over d=64, biased
    v = inorm(x @ Wv^T)
    dots_h = k_h^T v_h                [64, 64]  (contract over ALL N)
    u_h = q_h @ dots_h / N
    out = u @ Wo^T + bo               [N, 256]

Sharding: rows (N) split across 8 cores; only the tiny [B,H,64,64] dots
tensor is all-reduced.  Per-core dataflow (all shapes per batch):
    x [1024,256] --PE transpose--> xT [256,1024]
    k,v = (xT)^T @ W{k,v}T   (fp32r matmuls, moving dim 512)
    bn_stats -> per-(row,head) mean/rstd; normalized k~,v~ cast to fp16,
    with rstd_k*rstd_v folded into v~ (k,v only feed the dots matmul)
    dots: per head-pair packed [128,128] fp16 matmuls accumulated over row
    tiles (diagonal 64x64 blocks extracted afterwards)
    AllReduce(dots) across 8 cores
    qT = WqT^T @ xT (fp16 out), u^T = blockdiag(dots/N) @ qT (fp16)
    out = (uT)^T @ WoT + bo (fp32r), DMA out row-contiguous
"""

import os
import sys

import numpy as np

for _p in ("/opt/trn_rl_repo", os.path.expanduser("~/.axon_site/_ro/trn_rl_repo")):
    if os.path.isdir(_p) and _p not in sys.path:
        sys.path.insert(0, _p)

from contextlib import ExitStack

import concourse.bass as bass
import concourse.mybir as mybir
import concourse.tile as tile
from concourse import bacc
from concourse.bass_utils import run_bass_kernel_spmd
from concourse.masks import make_identity

F32 = mybir.dt.float32
F32R = mybir.dt.float32r
F16 = mybir.dt.float16

B, CIN = 4, 256
H, D = 8, 64
INNER, COUT = 512, 256
EPS = 1e-5
NCORES = 8
N_FULL = 8192
NPAIR = H // 2  # head pairs


def _build(n_chunk, n_full=N_FULL, ncores=NCORES):
    """Build the per-core SPMD Bass program. n_chunk rows per batch per core."""
    NT = n_chunk // 128  # 128-row tiles per batch
    nc = bacc.Bacc(
        "TRN2", target_bir_lowering=False, debug=False, num_devices=ncores)

    x_d = nc.declare_dram_parameter("x", [B, n_chunk, CIN], F32, isOutput=False)
    wq_d = nc.declare_dram_parameter("wq", [INNER, CIN], F32, isOutput=False)
    wk_d = nc.declare_dram_parameter("wk", [INNER, CIN], F32, isOutput=False)
    wv_d = nc.declare_dram_parameter("wv", [INNER, CIN], F32, isOutput=False)
    wo_d = nc.declare_dram_parameter("wo", [COUT, INNER], F32, isOutput=False)
    bo_d = nc.declare_dram_parameter("bo", [1, COUT], F32, isOutput=False)
    out_d = nc.declare_dram_parameter("out", [B, n_chunk, COUT], F32, isOutput=True)

    with ExitStack() as ctx:
        tc = ctx.enter_context(tile.TileContext(nc))
        _body(ctx, tc, nc, NT, n_full, ncores,
              x_d, wq_d, wk_d, wv_d, wo_d, bo_d, out_d)
    nc.compile()
    return nc


def _body(ctx, tc, nc, NT, n_full, ncores,
          x_d, wq_d, wk_d, wv_d, wo_d, bo_d, out_d):
    n_chunk = NT * 128

    # ---------------- pools ----------------
    # PSUM: 8 banks total. xpose(2) + kv(2) + dots(2) + big(2) = 8.
    xpose_ps = ctx.enter_context(tc.tile_pool(name="xpose_ps", bufs=1, space="PSUM"))
    kv_ps = ctx.enter_context(tc.tile_pool(name="kv_ps", bufs=3, space="PSUM"))
    dots_ps = ctx.enter_context(tc.tile_pool(name="dots_ps", bufs=2, space="PSUM"))
    big_ps = ctx.enter_context(tc.tile_pool(name="big_ps", bufs=2, space="PSUM"))

    consts = ctx.enter_context(tc.tile_pool(name="consts", bufs=1))
    wload = ctx.enter_context(tc.tile_pool(name="wload", bufs=2))
    x_pool = ctx.enter_context(tc.tile_pool(name="x_pool", bufs=4))
    xT_pool = ctx.enter_context(tc.tile_pool(name="xT_pool", bufs=2 * B))
    raw_pool = ctx.enter_context(tc.tile_pool(name="raw_pool", bufs=2))
    stats_pool = ctx.enter_context(tc.tile_pool(name="stats_pool", bufs=2))
    small_pool = ctx.enter_context(tc.tile_pool(name="small_pool", bufs=4))
    vtmp_pool = ctx.enter_context(tc.tile_pool(name="vtmp_pool", bufs=2))
    kt_pool = ctx.enter_context(tc.tile_pool(name="kt_pool", bufs=2 * NT))
    vt_pool = ctx.enter_context(tc.tile_pool(name="vt_pool", bufs=2 * NT))
    qT_pool = ctx.enter_context(tc.tile_pool(name="qT_pool", bufs=4 * B))
    uT_pool = ctx.enter_context(tc.tile_pool(name="uT_pool", bufs=8))
    bd_pool = ctx.enter_context(tc.tile_pool(name="bd_pool", bufs=8))
    out_pool = ctx.enter_context(tc.tile_pool(name="out_pool", bufs=6))
    dram = ctx.enter_context(tc.tile_pool(name="dram", bufs=1, space="DRAM"))

    # ---------------- constants / weights ----------------
    ident = consts.tile([128, 128], F16, tag="ident")
    make_identity(nc, ident[:])

    wq_t = [consts.tile([128, INNER], F16, tag=f"wq_t{c}", name=f"wq_t{c}") for c in range(2)]
    wkv_t = [consts.tile([128, 2 * INNER], F16, tag=f"wkv_t{c}", name=f"wkv_t{c}") for c in range(2)]
    wo_t = [consts.tile([128, COUT], F16, tag=f"wo_t{j}", name=f"wo_t{j}") for j in range(4)]

    def load_transposed(w_d, n_rows, store):
        # w_d: [n_rows, CIN] natural; store(ei, cs, psum[128c,128r]) writes dest.
        for ei in range(n_rows // 128):
            wn = wload.tile([128, CIN], F16, tag="wn")
            nc.gpsimd.dma_start(wn[:], w_d[ei * 128:(ei + 1) * 128, :])
            for cs in range(2):
                ps = xpose_ps.tile([128, 128], F16, tag="t")
                nc.tensor.transpose(ps[:], wn[:, cs * 128:(cs + 1) * 128], ident[:])
                store(ei, cs, ps)

    load_transposed(
        wq_d, INNER,
        lambda ei, cs, ps: nc.scalar.copy(wq_t[cs][:, ei * 128:(ei + 1) * 128], ps[:]))
    load_transposed(
        wk_d, INNER,
        lambda ei, cs, ps: nc.scalar.copy(wkv_t[cs][:, ei * 128:(ei + 1) * 128], ps[:]))
    load_transposed(
        wv_d, INNER,
        lambda ei, cs, ps: nc.scalar.copy(
            wkv_t[cs][:, INNER + ei * 128:INNER + (ei + 1) * 128], ps[:]))

    # WoT: Wo [COUT, INNER] -> wo_t[j] [128e, COUT]
    for oi in range(COUT // 128):
        wn = wload.tile([128, INNER], F16, tag="wn2")
        nc.gpsimd.dma_start(wn[:], wo_d[oi * 128:(oi + 1) * 128, :])
        for j in range(4):
            ps = xpose_ps.tile([128, 128], F16, tag="t")
            nc.tensor.transpose(ps[:], wn[:, j * 128:(j + 1) * 128], ident[:])
            nc.scalar.copy(wo_t[j][:, oi * 128:(oi + 1) * 128], ps[:])

    # bias broadcast [128, COUT] via ones outer product
    bo_sb = consts.tile([1, COUT], F32, tag="bo_sb")
    nc.sync.dma_start(bo_sb[:], bo_d[:])
    ones1 = consts.tile([1, 128], F32, tag="ones1")
    nc.gpsimd.memset(ones1[:], 1.0)
    bias_ps = big_ps.tile([128, 512], F32, tag="t")
    nc.tensor.matmul(bias_ps[:, :COUT], ones1[:], bo_sb[:], start=True, stop=True)
    bias_bc = consts.tile([128, COUT], F32, tag="bias_bc")
    nc.scalar.copy(bias_bc[:], bias_ps[:, :COUT])

    # per-head mean weights: msum_t[cs][c, 16] = sum_d wkv_t[cs][c, (kv,h,d)]
    m16_t = []
    for cs in range(2):
        msf = wload.tile([128, 16], F32, tag="msf", name=f"msf{cs}")
        nc.vector.reduce_sum(msf[:], wkv_t[cs][:].rearrange(
            "p (g d) -> p g d", d=D), axis=mybir.AxisListType.X)
        m16 = consts.tile([128, 16], F16, tag=f"m16_{cs}", name=f"m16_{cs}")
        nc.scalar.copy(m16[:], msf[:])
        m16_t.append(m16)

    # dots staging: [128, B * NPAIR * 64]
    dcols = B * NPAIR * 64
    dots_l = consts.tile([128, dcols], F32, tag="dots_l")
    dots_a = consts.tile([128, dcols], F32, tag="dots_a")

    xT_all = {}   # (b, cs) -> [128, n_chunk] f32
    kt_all = {}   # (b, nt) -> [128, 512] f16 (k - mean)
    vt_all = {}   # (b, nt) -> [128, 512] f16 ((v - mean) * rstd_k * rstd_v)

    # ---------------- phase 1: per-batch projections, norm, dots ----------------
    for b in range(B):
        # x load + transpose
        for cs in range(2):
            xT_all[(b, cs)] = xT_pool.tile([128, n_chunk], F16, tag="xT", name=f"xT_{b}_{cs}")
        for nt in range(NT):
            x_t = x_pool.tile([128, CIN], F16, tag="x")
            nc.gpsimd.dma_start(x_t[:], x_d[b, nt * 128:(nt + 1) * 128, :])
            for cs in range(2):
                ps = xpose_ps.tile([128, 128], F16, tag="t")
                nc.tensor.transpose(ps[:], x_t[:, cs * 128:(cs + 1) * 128], ident[:])
                nc.scalar.copy(xT_all[(b, cs)][:, nt * 128:(nt + 1) * 128], ps[:])

        # k,v projections + per-(row,head) sum / sum-of-squares + copy to sbuf
        kraw = raw_pool.tile([128, NT * 512], F16, tag="kraw")
        vraw = raw_pool.tile([128, NT * 512], F16, tag="vraw")
        ksum = stats_pool.tile([128, NT * 8], F32, tag="ksum")
        vsum = stats_pool.tile([128, NT * 8], F32, tag="vsum")
        ksq = stats_pool.tile([128, NT * 8], F32, tag="ksq")
        vsq = stats_pool.tile([128, NT * 8], F32, tag="vsq")
        ksumv = ksum.rearrange("p (t h) -> p t h", h=8)
        vsumv = vsum.rearrange("p (t h) -> p t h", h=8)
        ksqv = ksq.rearrange("p (t h) -> p t h", h=8)
        vsqv = vsq.rearrange("p (t h) -> p t h", h=8)
        for nt in range(NT):
            kps = kv_ps.tile([128, 512], F32, tag="t")
            vps = kv_ps.tile([128, 512], F32, tag="t")
            for cs in range(2):
                xT_sl = xT_all[(b, cs)][:, nt * 128:(nt + 1) * 128]
                nc.tensor.matmul(kps[:], xT_sl, wkv_t[cs][:, :INNER],
                                 start=(cs == 0), stop=(cs == 1))
                nc.tensor.matmul(vps[:], xT_sl, wkv_t[cs][:, INNER:],
                                 start=(cs == 0), stop=(cs == 1))
            mps = dots_ps.tile([128, 16], F32, tag="t", name="mps")
            for cs in range(2):
                xT_sl = xT_all[(b, cs)][:, nt * 128:(nt + 1) * 128]
                nc.tensor.matmul(mps[:], xT_sl, m16_t[cs][:],
                                 start=(cs == 0), stop=(cs == 1))
            nc.vector.tensor_copy(ksumv[:, nt, :], mps[:, 0:8])
            nc.vector.tensor_copy(vsumv[:, nt, :], mps[:, 8:16])
            kr_sl = kraw[:, nt * 512:(nt + 1) * 512]
            vr_sl = vraw[:, nt * 512:(nt + 1) * 512]
            nc.scalar.copy(kr_sl, kps[:])
            nc.scalar.copy(vr_sl, vps[:])
            sqk = vtmp_pool.tile([128, 512], F16, tag="sq", bufs=3)
            sqv = vtmp_pool.tile([128, 512], F16, tag="sq", bufs=3)
            nc.vector.tensor_tensor(sqk[:], kr_sl, kr_sl, op=mybir.AluOpType.mult)
            nc.vector.tensor_tensor(sqv[:], vr_sl, vr_sl, op=mybir.AluOpType.mult)
            nc.vector.reduce_sum(ksqv[:, nt, :],
                                 sqk.rearrange("p (h d) -> p h d", d=D),
                                 axis=mybir.AxisListType.X)
            nc.vector.reduce_sum(vsqv[:, nt, :],
                                 sqv.rearrange("p (h d) -> p h d", d=D),
                                 axis=mybir.AxisListType.X)

        # stats -> mean, rstd  (all [128, NT*8])
        def combine(sums, sumsq, tagp):
            mean = small_pool.tile([128, NT * 8], F32, tag=f"mean{tagp}")
            msq = small_pool.tile([128, NT * 8], F32, tag=f"msq{tagp}")
            var = small_pool.tile([128, NT * 8], F32, tag=f"var{tagp}")
            rstd = small_pool.tile([128, NT * 8], F32, tag=f"rstd{tagp}")
            nc.vector.tensor_scalar_mul(mean[:], sums[:], 1.0 / D)
            nc.vector.tensor_tensor(msq[:], mean[:], mean[:], op=mybir.AluOpType.mult)
            nc.vector.scalar_tensor_tensor(
                var[:], sumsq[:], 1.0 / D, msq[:],
                op0=mybir.AluOpType.mult, op1=mybir.AluOpType.subtract)
            nc.vector.tensor_scalar_add(var[:], var[:], EPS)
            nc.scalar.activation(var[:], var[:], mybir.ActivationFunctionType.Sqrt)
            nc.vector.reciprocal(rstd[:], var[:])
            return mean, rstd

        kmean, krstd = combine(ksum, ksq, "k")
        vmean, vrstd = combine(vsum, vsq, "v")
        w_sc = small_pool.tile([128, NT * 8], F32, tag="wsc")
        nc.vector.tensor_tensor(w_sc[:], krstd[:], vrstd[:], op=mybir.AluOpType.mult)

        kmv = kmean.rearrange("p (t h) -> p t h", h=8)
        vmv = vmean.rearrange("p (t h) -> p t h", h=8)
        wv_ = w_sc.rearrange("p (t h) -> p t h", h=8)
        krv = kraw.rearrange("p (t h d) -> p t h d", h=8, d=D)
        vrv = vraw.rearrange("p (t h d) -> p t h d", h=8, d=D)

        # apply: kt = k - mean (f16); vt = (v - mean) * w (f16)
        for nt in range(NT):
            kt = kt_pool.tile([128, 512], F16, tag="kt")
            vt = vt_pool.tile([128, 512], F16, tag="vt")
            kt_all[(b, nt)] = kt
            vt_all[(b, nt)] = vt
            ktv = kt.rearrange("p (h d) -> p h d", d=D)
            vtv = vt.rearrange("p (h d) -> p h d", d=D)
            nc.vector.tensor_tensor(
                ktv, krv[:, nt, :, :],
                kmv[:, nt, :].broadcast_to([128, 8, D]),
                op=mybir.AluOpType.subtract)
            vtmp = vtmp_pool.tile([128, 512], F16, tag="vtmp")
            vtmpv = vtmp.rearrange("p (h d) -> p h d", d=D)
            nc.vector.tensor_tensor(
                vtmpv, vrv[:, nt, :, :],
                vmv[:, nt, :].broadcast_to([128, 8, D]),
                op=mybir.AluOpType.subtract)
            nc.vector.tensor_tensor(
                vtv, vtmpv,
                wv_[:, nt, :].broadcast_to([128, 8, D]),
                op=mybir.AluOpType.mult)

        # dots: per head pair p, accumulate k_pair^T v_pair over row tiles
        for p in range(NPAIR):
            acc = dots_ps.tile([128, 128], F32, tag="t")
            for nt in range(NT):
                nc.tensor.matmul(
                    acc[:],
                    kt_all[(b, nt)][:, p * 128:(p + 1) * 128],
                    vt_all[(b, nt)][:, p * 128:(p + 1) * 128],
                    start=(nt == 0), stop=(nt == NT - 1))
            col = (b * NPAIR + p) * 64
            nc.vector.tensor_copy(dots_l[0:64, col:col + 64], acc[0:64, 0:64])
            nc.vector.tensor_copy(dots_l[64:128, col:col + 64], acc[64:128, 64:128])

        bcols = NPAIR * 64
        cc_in = dram.tile([128, bcols], F32, tag="cc_in", bufs=B, name=f"cc_in{b}")
        cc_out = dram.tile([128, bcols], F32, tag="cc_out", bufs=B, name=f"cc_out{b}")
        bsl = slice(b * bcols, (b + 1) * bcols)
        nc.sync.dma_start(cc_in[:], dots_l[:, bsl])
        nc.gpsimd.collective_compute(
            "AllReduce", mybir.AluOpType.add,
            replica_groups=[list(range(ncores))],
            ins=[cc_in.opt()], outs=[cc_out.opt()])
        nc.sync.dma_start(dots_a[:, bsl], cc_out[:])


    # ---------------- phase 3: q projection (overlaps all-reduce) ----------------
    qT_all = {}
    for b in range(B):
        for j in range(4):
            qT = qT_pool.tile([128, n_chunk], F16, tag="qT")
            qT_all[(b, j)] = qT
            for ch in range(n_chunk // 512):
                qps = dots_ps.tile([128, 512], F32, tag="t", name="qps")
                for cs in range(2):
                    nc.tensor.matmul(
                        qps[:],
                        wq_t[cs][:, j * 128:(j + 1) * 128],
                        xT_all[(b, cs)][:, ch * 512:(ch + 1) * 512],
                        start=(cs == 0), stop=(cs == 1))
                nc.scalar.copy(qT[:, ch * 512:(ch + 1) * 512], qps[:])

    # ---------------- phase 4: u = blockdiag(dots/N) @ qT, out projection ----------------
    for b in range(B):
        uT_b = []
        for j in range(4):
            bd = bd_pool.tile([128, 128], F16, tag="bd")
            nc.gpsimd.memset(bd[:], 0.0)
            col = (b * NPAIR + j) * 64
            nc.scalar.activation(bd[0:64, 0:64], dots_a[0:64, col:col + 64],
                                 mybir.ActivationFunctionType.Copy, scale=1.0 / n_full)
            nc.scalar.activation(bd[64:128, 64:128], dots_a[64:128, col:col + 64],
                                 mybir.ActivationFunctionType.Copy, scale=1.0 / n_full)
            uT = uT_pool.tile([128, n_chunk], F16, tag="uT")
            uT_b.append(uT)
            for ch in range(n_chunk // 512):
                ups = kv_ps.tile([128, 512], F32, tag="t", name="ups")
                nc.tensor.matmul(ups[:], bd[:],
                                 qT_all[(b, j)][:, ch * 512:(ch + 1) * 512],
                                 start=True, stop=True)
                nc.scalar.copy(uT[:, ch * 512:(ch + 1) * 512], ups[:])

        for nt in range(NT):
            ops = big_ps.tile([128, 512], F32, tag="t")
            for j in range(4):
                nc.tensor.matmul(
                    ops[:, :COUT],
                    uT_b[j][:, nt * 128:(nt + 1) * 128],
                    wo_t[j][:],
                    start=(j == 0), stop=(j == 3))
            osb = out_pool.tile([128, COUT], F32, tag="osb")
            nc.vector.tensor_tensor(osb[:], ops[:, :COUT], bias_bc[:],
                                    op=mybir.AluOpType.add)
            nc.sync.dma_start(out_d[b, nt * 128:(nt + 1) * 128, :], osb[:])


_NC_CACHE = {}


def _get_nc(n_chunk, n_full, ncores):
    key = (n_chunk, n_full, ncores)
    if key not in _NC_CACHE:
        _NC_CACHE[key] = _build(n_chunk, n_full, ncores)
    return _NC_CACHE[key]


def _make_in_maps(u_x, Wq, Wk, Wv, Wo, bo, ncores):
    n = u_x.shape[1]
    n_chunk = n // ncores
    wq = np.ascontiguousarray(np.asarray(Wq, np.float32))
    wk = np.ascontiguousarray(np.asarray(Wk, np.float32))
    wv = np.ascontiguousarray(np.asarray(Wv, np.float32))
    wo = np.ascontiguousarray(np.asarray(Wo, np.float32))
    bo2 = np.ascontiguousarray(np.asarray(bo, np.float32).reshape(1, -1))
    u_x = np.asarray(u_x, np.float32)
    maps = []
    for c in range(ncores):
        maps.append({
            "x": np.ascontiguousarray(u_x[:, c * n_chunk:(c + 1) * n_chunk, :]),
            "wq": wq, "wk": wk, "wv": wv, "wo": wo, "bo": bo2,
        })
    return maps, n_chunk


def _install_ntff_hook():
    """Provide antenv.axon_hooks (missing in this image) so trace=True works."""
    import types
    try:
        from antenv.axon_hooks import get_axon_ntff_profile_hook  # noqa: F401
        return  # real module present
    except ImportError:
        pass
    try:
        import antenv
        mod = types.ModuleType("antenv.axon_hooks")
        _state = {"hook": None}
        mod.set_axon_ntff_profile_hook = lambda h: _state.__setitem__("hook", h)
        mod.get_axon_ntff_profile_hook = lambda: _state["hook"]
        sys.modules["antenv.axon_hooks"] = mod
        antenv.axon_hooks = mod
        boot_dir = "/root/.axon_site/trn_agent_boot"
        if boot_dir not in sys.path and os.path.isdir(boot_dir):
            sys.path.insert(0, boot_dir)
        import trn_boot
        so_path = "/opt/axon/libaxon_pjrt.so"
        if os.path.exists(so_path):
            hook = trn_boot._ntff_profile_via_ctypes(so_path)
            if hook is not None:
                mod.set_axon_ntff_profile_hook(hook)
    except Exception as e:  # tracing is best-effort; never break the run path
        print(f"ntff hook install failed: {e}", file=sys.stderr)


def run(u_x, Wq, Wk, Wv, Wo, bo, n_full=None, ncores=NCORES, trace=False,
        tmpdir=None):
    if trace:
        _install_ntff_hook()
    n = u_x.shape[1]
    if n_full is None:
        n_full = n
    in_maps, n_chunk = _make_in_maps(u_x, Wq, Wk, Wv, Wo, bo, ncores)
    nc = _get_nc(n_chunk, n_full, ncores)
    res = run_bass_kernel_spmd(nc, in_maps, list(range(ncores)), trace=trace,
                               tmpdir=tmpdir)
    outs = [np.asarray(res.results[c]["out"]) for c in range(ncores)]
    full = np.concatenate(outs, axis=1).astype(np.float32)
    return full, res


def kernel(u_x, pos_x=None, Wq=None, Wk=None, Wv=None, Wo=None, bo=None):
    full, _ = run(np.asarray(u_x, np.float32), Wq, Wk, Wv, Wo, bo)
    return full



# revision 10
# speedup vs baseline: 1.1822x; 1.1822x over previous
"""Trainium2 Bass kernel for nn_AttentionKernelIntegral (linear attention).

Math (per batch b, head h):
    q = x @ Wq^T                      [N, 512]  (no norm)
    k = inorm(x @ Wk^T)               per-(n,h) mean/var over d=64, biased
    v = inorm(x @ Wv^T)
    dots_h = k_h^T v_h                [64, 64]  (contract over ALL N)
    u_h = q_h @ dots_h / N
    out = u @ Wo^T + bo               [N, 256]

Key transforms vs the straightforward dataflow:
  * Mean subtraction folds into the weights: center Wk/Wv columns per head
    (InstanceNorm mean of a linear map = linear map with centered weights).
  * The whole q/dots/out chain folds into a per-batch [256,256] matrix:
        W3_b = (Wq/64)^T @ blockdiag(dots_b/128) @ Wo^T     (1/64*1/128=1/N)
        out  = x @ W3_b + bo
    so only k/v are ever materialized at [N,512] width.
  * rstd_k*rstd_v is computed as one rsqrt((vark+eps)*(varv+eps)) and
    applied to the k side only; v stays raw (centered) fp16.

Sharding: rows (N) split across 8 cores; only the [B,H,64,64] dots tensor
is all-reduced (fp16, scaled by 1/128 for range).
"""

import os
import sys

import numpy as np

for _p in ("/opt/trn_rl_repo", os.path.expanduser("~/.axon_site/_ro/trn_rl_repo")):
    if os.path.isdir(_p) and _p not in sys.path:
        sys.path.insert(0, _p)

from contextlib import ExitStack

import concourse.bass as bass
import concourse.mybir as mybir
import concourse.tile as tile
from concourse import bacc
from concourse.bass_utils import run_bass_kernel_spmd
from concourse.masks import make_identity

F32 = mybir.dt.float32
F16 = mybir.dt.float16

B, CIN = 4, 256
H, D = 8, 64
INNER, COUT = 512, 256
EPS = 1e-5
NCORES = 8
N_FULL = 8192
NPAIR = H // 2  # head pairs packed into 128-wide dots matmuls
DOTS_SCALE = 1.0 / 128.0  # dots staged as dots/128; Wq staged as Wq*(128/N)


def _build(n_chunk, n_full=N_FULL, ncores=NCORES):
    """Build the per-core SPMD Bass program. n_chunk rows per batch per core."""
    NT = n_chunk // 128  # 128-row tiles per batch
    nc = bacc.Bacc(
        "TRN2", target_bir_lowering=False, debug=False, num_devices=ncores)

    x_d = nc.declare_dram_parameter("x", [B, n_chunk, CIN], F32, isOutput=False)
    wq_d = nc.declare_dram_parameter("wq", [INNER, CIN], F32, isOutput=False)
    wk_d = nc.declare_dram_parameter("wk", [INNER, CIN], F32, isOutput=False)
    wv_d = nc.declare_dram_parameter("wv", [INNER, CIN], F32, isOutput=False)
    wo_d = nc.declare_dram_parameter("wo", [COUT, INNER], F32, isOutput=False)
    bo_d = nc.declare_dram_parameter("bo", [1, COUT], F32, isOutput=False)
    out_d = nc.declare_dram_parameter("out", [B, n_chunk, COUT], F32, isOutput=True)

    with ExitStack() as ctx:
        tc = ctx.enter_context(tile.TileContext(nc))
        _body(ctx, tc, nc, NT, n_full, ncores,
              x_d, wq_d, wk_d, wv_d, wo_d, bo_d, out_d)
    nc.compile()
    return nc


def _body(ctx, tc, nc, NT, n_full, ncores,
          x_d, wq_d, wk_d, wv_d, wo_d, bo_d, out_d):
    n_chunk = NT * 128
    AF = mybir.ActivationFunctionType
    OP = mybir.AluOpType

    # ---------------- pools ----------------
    # PSUM: 8 banks. xpose(1) + kv(3) + dots(2) + big(2) = 8.
    xpose_ps = ctx.enter_context(tc.tile_pool(name="xpose_ps", bufs=1, space="PSUM"))
    kv_ps = ctx.enter_context(tc.tile_pool(name="kv_ps", bufs=3, space="PSUM"))
    dots_ps = ctx.enter_context(tc.tile_pool(name="dots_ps", bufs=2, space="PSUM"))
    big_ps = ctx.enter_context(tc.tile_pool(name="big_ps", bufs=2, space="PSUM"))

    consts = ctx.enter_context(tc.tile_pool(name="consts", bufs=1))
    wload = ctx.enter_context(tc.tile_pool(name="wload", bufs=2))
    xload_pool = ctx.enter_context(tc.tile_pool(name="xload", bufs=B))
    xT_pool = ctx.enter_context(tc.tile_pool(name="xT_pool", bufs=2 * B))
    kraw_pool = ctx.enter_context(tc.tile_pool(name="kraw", bufs=2 * NT))
    stats_pool = ctx.enter_context(tc.tile_pool(name="stats_pool", bufs=2))
    small_pool = ctx.enter_context(tc.tile_pool(name="small_pool", bufs=4))
    kt_pool = ctx.enter_context(tc.tile_pool(name="kt_pool", bufs=2 * NT))
    vt_pool = ctx.enter_context(tc.tile_pool(name="vt_pool", bufs=2 * NT))
    d16_pool = ctx.enter_context(tc.tile_pool(name="d16_pool", bufs=2))
    w2_pool = ctx.enter_context(tc.tile_pool(name="w2_pool", bufs=8))
    out_pool = ctx.enter_context(tc.tile_pool(name="out_pool", bufs=6))
    dram = ctx.enter_context(tc.tile_pool(name="dram", bufs=1, space="DRAM"))

    # ---------------- constants / weights ----------------
    ident = consts.tile([128, 128], F16, tag="ident")
    make_identity(nc, ident[:])

    # wkv_t[cs]: [128c, k(512) | v(512)] fp16, transposed from natural Wk/Wv
    wkv_t = [consts.tile([128, 2 * INNER], F16, tag=f"wkv_t{c}", name=f"wkv_t{c}")
             for c in range(2)]

    def load_transposed(w_d, n_rows, store):
        for ei in range(n_rows // 128):
            wn = wload.tile([128, CIN], F16, tag="wn")
            nc.gpsimd.dma_start(wn[:], w_d[ei * 128:(ei + 1) * 128, :])
            for cs in range(2):
                ps = xpose_ps.tile([128, 128], F16, tag="t")
                nc.tensor.transpose(ps[:], wn[:, cs * 128:(cs + 1) * 128], ident[:])
                store(ei, cs, ps)

    load_transposed(
        wk_d, INNER,
        lambda ei, cs, ps: nc.scalar.copy(wkv_t[cs][:, ei * 128:(ei + 1) * 128], ps[:]))
    load_transposed(
        wv_d, INNER,
        lambda ei, cs, ps: nc.scalar.copy(
            wkv_t[cs][:, INNER + ei * 128:INNER + (ei + 1) * 128], ps[:]))

    # center Wk/Wv per head over d (folds InstanceNorm mean into the weights)
    wkv_c = [consts.tile([128, 2 * INNER], F16, tag=f"wkv_c{c}", name=f"wkv_c{c}")
             for c in range(2)]
    for cs in range(2):
        wv_view = wkv_t[cs][:].rearrange("p (g d) -> p g d", d=D)
        msum = small_pool.tile([128, 16], F32, tag="msum")
        nc.vector.reduce_sum(msum[:], wv_view, axis=mybir.AxisListType.X)
        m16 = small_pool.tile([128, 16], F16, tag="m16")
        nc.vector.tensor_scalar_mul(m16[:], msum[:], 1.0 / D)
        nc.vector.tensor_tensor(
            wkv_c[cs][:].rearrange("p (g d) -> p g d", d=D),
            wv_view, m16[:].broadcast_to([128, 16, D]), op=OP.subtract)

    # wq natural (NOT transposed), scaled by 128/n_full: wq_nat[j] [128i, 256c]
    wq_nat = []
    for j in range(4):
        wqr = wload.tile([128, CIN], F16, tag="wqr")
        nc.gpsimd.dma_start(wqr[:], wq_d[j * 128:(j + 1) * 128, :])
        wqn = consts.tile([128, CIN], F16, tag=f"wq_nat{j}", name=f"wq_nat{j}")
        nc.scalar.activation(wqn[:], wqr[:], AF.Copy, scale=128.0 / n_full)
        wq_nat.append(wqn)

    # WoT: Wo [COUT, INNER] -> wo_t[j] [128i, COUT]
    wo_t = [consts.tile([128, COUT], F16, tag=f"wo_t{j}", name=f"wo_t{j}") for j in range(4)]
    for oi in range(COUT // 128):
        wn = wload.tile([128, INNER], F16, tag="wn2")
        nc.gpsimd.dma_start(wn[:], wo_d[oi * 128:(oi + 1) * 128, :])
        for j in range(4):
            ps = xpose_ps.tile([128, 128], F16, tag="t")
            nc.tensor.transpose(ps[:], wn[:, j * 128:(j + 1) * 128], ident[:])
            nc.scalar.copy(wo_t[j][:, oi * 128:(oi + 1) * 128], ps[:])

    # bias broadcast [128, COUT] via ones outer product
    bo_sb = consts.tile([1, COUT], F32, tag="bo_sb")
    nc.sync.dma_start(bo_sb[:], bo_d[:])
    ones1 = consts.tile([1, 128], F32, tag="ones1")
    nc.gpsimd.memset(ones1[:], 1.0)
    bias_ps = big_ps.tile([128, 512], F32, tag="t")
    nc.tensor.matmul(bias_ps[:, :COUT], ones1[:], bo_sb[:], start=True, stop=True)
    bias_bc = consts.tile([128, COUT], F32, tag="bias_bc")
    nc.scalar.copy(bias_bc[:], bias_ps[:, :COUT])

    # batched x loads (f32 dram -> f16 sbuf), one DMA per batch, issued upfront
    xload = []
    for b in range(B):
        xl = xload_pool.tile([128, NT, CIN], F16, tag="xl", name=f"xload{b}")
        src = x_d[b, :, :].rearrange("(t p) c -> p t c", p=128)
        nc.gpsimd.dma_start(xl[:], src)
        xload.append(xl)

    xT_all = {}     # (b, cs) -> [128c, n_chunk] f16
    kt_all = {}     # (b, nt) -> [128, 512] f16  (centered k * w)
    vt_all = {}     # (b, nt) -> [128, 512] f16  (centered v, raw)
    kraw_all = {}   # (b, nt) -> [128, 512] f16  (centered k, raw)
    bn_all = {}     # b -> [128, NT*2*8] f32 per-(row,head) sum-of-squares (k|v)
    w16_all = {}    # b -> [128, NT*8] f16 combined rstd_k*rstd_v
    dots16_all = {}  # b -> [128, NPAIR*64] f16 staged dots/128
    dots_a16 = {}   # b -> [128, NPAIR*64] f16 allreduced

    # ---------------- stage A: projections + bn stats (per batch) ----------------
    def stage_a(b):
        for cs in range(2):
            xT_all[(b, cs)] = xT_pool.tile([128, n_chunk], F16, tag="xT",
                                           name=f"xT_{b}_{cs}")
        for nt in range(NT):
            for cs in range(2):
                ps = xpose_ps.tile([128, 128], F16, tag="t")
                nc.tensor.transpose(
                    ps[:], xload[b][:, nt, cs * 128:(cs + 1) * 128], ident[:])
                nc.scalar.copy(xT_all[(b, cs)][:, nt * 128:(nt + 1) * 128], ps[:])

        sqred = stats_pool.tile([128, NT * 16], F32, tag="sqred", name=f"sqred{b}")
        bn_all[b] = sqred
        sqredv = sqred.rearrange("p (t g h) -> p t g h", g=2, h=8)
        for nt in range(NT):
            kps = kv_ps.tile([128, 512], F32, tag="t")
            vps = kv_ps.tile([128, 512], F32, tag="t")
            for cs in range(2):
                xT_sl = xT_all[(b, cs)][:, nt * 128:(nt + 1) * 128]
                nc.tensor.matmul(kps[:], xT_sl, wkv_c[cs][:, :INNER],
                                 start=(cs == 0), stop=(cs == 1))
                nc.tensor.matmul(vps[:], xT_sl, wkv_c[cs][:, INNER:],
                                 start=(cs == 0), stop=(cs == 1))
            kr = kraw_pool.tile([128, 512], F16, tag="kr")
            vt = vt_pool.tile([128, 512], F16, tag="vt")
            kraw_all[(b, nt)] = kr
            vt_all[(b, nt)] = vt
            nc.scalar.copy(kr[:], kps[:])
            nc.scalar.copy(vt[:], vps[:])
            # squares (DVE f16 2x); one shared wide reduce (DVE)
            sq = stats_pool.tile([128, 1024], F16, tag="sq", bufs=3)
            nc.vector.tensor_tensor(sq[:, :512], kr[:], kr[:], op=OP.mult)
            nc.vector.tensor_tensor(sq[:, 512:], vt[:], vt[:], op=OP.mult)
            nc.vector.reduce_sum(
                sqredv[:, nt, :, :],
                sq[:].rearrange("p (g h d) -> p g h d", h=8, d=D),
                axis=mybir.AxisListType.X)

    # ---------------- stage B: stats combine, kt scale, dots, allreduce ----------
    def stage_b(b):
        # w = rstd_k*rstd_v = 1/sqrt((ksq/64+eps)*(vsq/64+eps))
        #   = 1/sqrt((ksq+64eps)*(vsq+64eps)/4096)   (scale folded into Sqrt)
        sqredv = bn_all[b].rearrange("p (t g h) -> p t g h", g=2, h=8)
        ksq, vsq = sqredv[:, :, 0, :], sqredv[:, :, 1, :]
        ve = small_pool.tile([128, NT * 8], F32, tag="ve")
        nc.vector.tensor_scalar_add(
            ve[:].rearrange("p (t h) -> p t h", h=8), vsq, D * EPS)
        prod = small_pool.tile([128, NT * 8], F32, tag="prod")
        nc.vector.scalar_tensor_tensor(
            prod[:].rearrange("p (t h) -> p t h", h=8),
            ksq, D * EPS, ve[:].rearrange("p (t h) -> p t h", h=8),
            op0=OP.add, op1=OP.mult)
        nc.scalar.activation(prod[:], prod[:], AF.Sqrt, scale=1.0 / (D * D))
        wsc = small_pool.tile([128, NT * 8], F32, tag="wsc")
        nc.vector.reciprocal(wsc[:], prod[:])
        w16 = small_pool.tile([128, NT * 8], F16, tag="w16")
        nc.vector.tensor_copy(w16[:], wsc[:])
        w16_all[b] = w16
        w16v = w16.rearrange("p (t h) -> p t h", h=8)

        for nt in range(NT):
            kt = kt_pool.tile([128, 512], F16, tag="kt")
            kt_all[(b, nt)] = kt
            nc.gpsimd.tensor_tensor(
                kt[:].rearrange("p (h d) -> p h d", d=D),
                kraw_all[(b, nt)][:].rearrange("p (h d) -> p h d", d=D),
                w16v[:, nt, :].broadcast_to([128, 8, D]),
                op=OP.mult)

        # dots: per head pair, accumulate kt^T vt over row tiles
        d16 = d16_pool.tile([128, NPAIR * 64], F16, tag="d16", name=f"d16_{b}")
        dots16_all[b] = d16
        for p in range(NPAIR):
            acc = dots_ps.tile([128, 128], F32, tag="t")
            for nt in range(NT):
                nc.tensor.matmul(
                    acc[:],
                    kt_all[(b, nt)][:, p * 128:(p + 1) * 128],
                    vt_all[(b, nt)][:, p * 128:(p + 1) * 128],
                    start=(nt == 0), stop=(nt == NT - 1))
            col = p * 64
            nc.scalar.activation(d16[0:64, col:col + 64], acc[0:64, 0:64],
                                 AF.Copy, scale=DOTS_SCALE)
            nc.scalar.activation(d16[64:128, col:col + 64], acc[64:128, 64:128],
                                 AF.Copy, scale=DOTS_SCALE)

        bcols = NPAIR * 64
        cc_in = dram.tile([128, bcols], F16, tag="cc_in", bufs=B, name=f"cc_in{b}")
        cc_out = dram.tile([128, bcols], F16, tag="cc_out", bufs=B, name=f"cc_out{b}")
        nc.sync.dma_start(cc_in[:], d16[:])
        nc.gpsimd.collective_compute(
            "AllReduce", OP.add,
            replica_groups=[list(range(ncores))],
            ins=[cc_in.opt()], outs=[cc_out.opt()])
        da = d16_pool.tile([128, bcols], F16, tag="da", name=f"da_{b}")
        dots_a16[b] = da
        nc.sync.dma_start(da[:], cc_out[:])

    # ---------------- phase 2: compose W3 = Wq'^T dots' Wo^T, out = x W3 + bo ----
    def phase2(b):
        da = dots_a16[b]
        # W2T[j] [128i, 256c]: per head h, W2T rows h*64+e = dots_h^T @ Wq_h
        w2t16 = []
        for j in range(4):
            ps = big_ps.tile([128, 512], F32, tag="t")
            nc.tensor.matmul(ps[0:64, :CIN], da[0:64, j * 64:(j + 1) * 64],
                             wq_nat[j][0:64, :], start=True, stop=True)
            nc.tensor.matmul(ps[64:128, :CIN], da[64:128, j * 64:(j + 1) * 64],
                             wq_nat[j][64:128, :], start=True, stop=True)
            w2 = w2_pool.tile([128, CIN], F16, tag="w2")
            nc.scalar.copy(w2[:], ps[:, :CIN])
            w2t16.append(w2)
        # W3[cs] [128c, 256o] = sum_j W2T[j][:, cs]^T @ WoT[j]
        w3_16 = []
        for cs in range(2):
            ps = big_ps.tile([128, 512], F32, tag="t")
            for j in range(4):
                nc.tensor.matmul(ps[:, :COUT],
                                 w2t16[j][:, cs * 128:(cs + 1) * 128],
                                 wo_t[j][:], start=(j == 0), stop=(j == 3))
            w3 = w2_pool.tile([128, COUT], F16, tag="w3")
            nc.scalar.copy(w3[:], ps[:, :COUT])
            w3_16.append(w3)
        # out rows: out = xT^T @ W3 + bias
        for nt in range(NT):
            ops = big_ps.tile([128, 512], F32, tag="t")
            for cs in range(2):
                nc.tensor.matmul(ops[:, :COUT],
                                 xT_all[(b, cs)][:, nt * 128:(nt + 1) * 128],
                                 w3_16[cs][:], start=(cs == 0), stop=(cs == 1))
            osb = out_pool.tile([128, COUT], F32, tag="osb")
            nc.vector.tensor_tensor(osb[:], ops[:, :COUT], bias_bc[:], op=OP.add)
            nc.sync.dma_start(out_d[b, nt * 128:(nt + 1) * 128, :], osb[:])

    # ---------------- schedule: software-pipeline stage A/B, then phase 2 -------
    for b in range(B):
        stage_a(b)
        if b > 0:
            stage_b(b - 1)
    stage_b(B - 1)
    for b in range(B):
        phase2(b)


_NC_CACHE = {}


def _get_nc(n_chunk, n_full, ncores):
    key = (n_chunk, n_full, ncores)
    if key not in _NC_CACHE:
        _NC_CACHE[key] = _build(n_chunk, n_full, ncores)
    return _NC_CACHE[key]


def _make_in_maps(u_x, Wq, Wk, Wv, Wo, bo, ncores):
    n = u_x.shape[1]
    n_chunk = n // ncores
    wq = np.ascontiguousarray(np.asarray(Wq, np.float32))
    wk = np.ascontiguousarray(np.asarray(Wk, np.float32))
    wv = np.ascontiguousarray(np.asarray(Wv, np.float32))
    wo = np.ascontiguousarray(np.asarray(Wo, np.float32))
    bo2 = np.ascontiguousarray(np.asarray(bo, np.float32).reshape(1, -1))
    u_x = np.asarray(u_x, np.float32)
    maps = []
    for c in range(ncores):
        maps.append({
            "x": np.ascontiguousarray(u_x[:, c * n_chunk:(c + 1) * n_chunk, :]),
            "wq": wq, "wk": wk, "wv": wv, "wo": wo, "bo": bo2,
        })
    return maps, n_chunk


def _install_ntff_hook():
    """Provide antenv.axon_hooks (missing in this image) so trace=True works."""
    import types
    try:
        from antenv.axon_hooks import get_axon_ntff_profile_hook  # noqa: F401
        return  # real module present
    except ImportError:
        pass
    try:
        import antenv
        mod = types.ModuleType("antenv.axon_hooks")
        _state = {"hook": None}
        mod.set_axon_ntff_profile_hook = lambda h: _state.__setitem__("hook", h)
        mod.get_axon_ntff_profile_hook = lambda: _state["hook"]
        sys.modules["antenv.axon_hooks"] = mod
        antenv.axon_hooks = mod
        boot_dir = "/root/.axon_site/trn_agent_boot"
        if boot_dir not in sys.path and os.path.isdir(boot_dir):
            sys.path.insert(0, boot_dir)
        import trn_boot
        so_path = "/opt/axon/libaxon_pjrt.so"
        if os.path.exists(so_path):
            hook = trn_boot._ntff_profile_via_ctypes(so_path)
            if hook is not None:
                mod.set_axon_ntff_profile_hook(hook)
    except Exception as e:  # tracing is best-effort; never break the run path
        print(f"ntff hook install failed: {e}", file=sys.stderr)


def run(u_x, Wq, Wk, Wv, Wo, bo, n_full=None, ncores=NCORES, trace=False,
        tmpdir=None):
    if trace:
        _install_ntff_hook()
    n = u_x.shape[1]
    if n_full is None:
        n_full = n
    in_maps, n_chunk = _make_in_maps(u_x, Wq, Wk, Wv, Wo, bo, ncores)
    nc = _get_nc(n_chunk, n_full, ncores)
    res = run_bass_kernel_spmd(nc, in_maps, list(range(ncores)), trace=trace,
                               tmpdir=tmpdir)
    outs = [np.asarray(res.results[c]["out"]) for c in range(ncores)]
    full = np.concatenate(outs, axis=1).astype(np.float32)
    return full, res


def kernel(u_x, pos_x=None, Wq=None, Wk=None, Wv=None, Wo=None, bo=None):
    full, _ = run(np.asarray(u_x, np.float32), Wq, Wk, Wv, Wo, bo)
    return full
